# revision 29
# baseline (speedup 1.0000x reference)
"""Trainium2 Bass kernel for a seq2seq GRU (BiGRU encoder + teacher-forced GRU decoder).

Model (hardcoded): B=4096, S=T=16, V=128, E=256, H=512, fp32 in/out.
Sharding: pure data parallel — batch split across 8 NeuronCores, weights replicated.

Device-side design (per core, Bc = 512 words):
  * Everything lives in transposed [feature, batch] layout, so every weight
    matrix is consumed by the PE in its natural layout and no on-device
    transposes are needed anywhere.
  * All matmul operands are float32r (fp32 storage, full-rate PE mode,
    ~1.5e-4 matmul rel err vs 4x-slower exact fp32).
  * Embedding lookup + input projection are fused into one-hot matmuls
    against an on-device table xtab = emb @ Wx ([V=128, 3H]).
  * GRU step, engineered for a short cross-engine critical path:
      R banks first (PE) -> sigmoid chunks (ACT) while Z fills,
      v = r*hh, w = v+xh chunked (DVE) racing the HH/XH fills,
      c = tanh(w) chunked (ACT),
      h' = z*h + (1-z)*c with zh = z*h computed off-path and
      1-z = sigmoid(-a_z) free on the ACT affine input.
  * Encoder fwd/bwd chains interleave; decoder runs two batch halves
    (N=256) interleaved. PSUM: 2 x [128,4,W] tiles in flight.
  * Decoder logits [V,B] go to a [T,V,B] DRAM staging buffer; the final
    [B,T,V] transpose happens on host after gathering.
"""

import numpy as np

import concourse.bass as bass
import concourse.bacc as bacc
import concourse.mybir as mybir
from concourse.tile import TileContext
from concourse.bass_utils import run_bass_kernel_spmd

F32 = mybir.dt.float32
F32R = mybir.dt.float32r
AF = mybir.ActivationFunctionType
OP = mybir.AluOpType

P = 128
NCORES = 8
B, S, T = 4096, 16, 16
V, E, H = 128, 256, 512
BC = B // NCORES          # 512 words per core
KH = H // P               # 4 k-tiles of the hidden dim
KE = E // P               # 2 k-tiles of the embedding dim
H3 = 3 * H                # 1536
BOW = 1

# Stash of the most recent BassKernelResults (test.py reads it).
LAST_RESULT = None
_CACHED_NC = None


def _gru_step(nc, ps, gp, Wh_sb, xtab_sb, oh_ap, h_sb, h_new, W, tagp, h_mm=None):
    """One GRU step in transposed layout. See module docstring.

    PSUM tiles are [128, 2, W] (half a gate-group) so the pool rotates at
    2-bank granularity and the PE never waits long for a free slot.
    """
    first = h_sb is None
    G = tagp + "g"
    if h_mm is None:
        h_mm = [] if first else [h_sb]

    def fill(col0, with_h=True, with_oh=True):
        # two psum half-tiles covering out rows [col0 ... col0+4P)
        halves = []
        for hp in range(2):
            t = ps.tile([P, 2, W], F32, tag="ps", name="psh")
            for jj in range(2):
                m = col0 + (2 * hp + jj) * P
                if with_h:
                    for hi, hmm in enumerate(h_mm):
                        for k in range(KH):
                            last = hi == len(h_mm) - 1 and k == KH - 1
                            nc.tensor.matmul(t[:, jj, :], Wh_sb[:, k, m:m + P],
                                             hmm[:, k, :], start=(hi == 0 and k == 0),
                                             stop=(not with_oh and last))
                if with_oh:
                    nc.tensor.matmul(t[:, jj, :], xtab_sb[:, m:m + P], oh_ap,
                                     start=not with_h, stop=True)
            halves.append(t)
        return halves

    if not first:
        # ---- R gate first: v = r*hh is on the critical path ----
        rp = fill(H)
        r_sb = gp.tile([P, KH, W], F32, tag=G, name="r_sb")
        for hp in range(2):
            for jj in range(2):
                nc.scalar.activation(r_sb[:, 2 * hp + jj, :], rp[hp][:, jj, :], AF.Sigmoid)

    # ---- Z gate: z (update) and zbar = 1-z = sigmoid(-a_z) ----
    zph = fill(0, with_h=not first)
    zbar_sb = gp.tile([P, KH, W], F32, tag=G, name="zbar_sb")
    for hp in range(2):
        nc.scalar.activation(zbar_sb[:, 2 * hp:2 * hp + 2, :], zph[hp][:], AF.Sigmoid,
                             scale=-1.0)
    if not first:
        z_sb = gp.tile([P, KH, W], F32, tag=G, name="z_sb")
        for hp in range(2):
            nc.scalar.activation(z_sb[:, 2 * hp:2 * hp + 2, :], zph[hp][:], AF.Sigmoid)

    if not first:
        # ---- HH = h @ Wh_h (kept separate from xh: reset_after GRU) ----
        hh = fill(2 * H, with_oh=False)
    # ---- XH = one-hot xh (+bx_h folded into the table) ----
    xhh = fill(2 * H, with_h=False)

    c_sb = gp.tile([P, KH, W], F32, tag=G, name="c_sb")
    if not first:
        # chunked v = r*hh (then in-place += xh) then tanh, racing PE fills
        v_sb = gp.tile([P, KH, W], F32, tag=G, name="v_sb")
        for hp in range(2):
            for jj in range(2):
                nc.vector.tensor_tensor(v_sb[:, 2 * hp + jj, :], r_sb[:, 2 * hp + jj, :],
                                        hh[hp][:, jj, :], OP.mult)
        for hp in range(2):
            nc.vector.tensor_tensor(v_sb[:, 2 * hp:2 * hp + 2, :], v_sb[:, 2 * hp:2 * hp + 2, :],
                                    xhh[hp][:], OP.add)
        for hp in range(2):
            for jj in range(2):
                nc.scalar.activation(c_sb[:, 2 * hp + jj, :], v_sb[:, 2 * hp + jj, :], AF.Tanh)
        # off-path: zh = z*h ; then u = zbar*c (in-place on c) ; h' = zh+u
        zh_sb = gp.tile([P, KH, W], F32, tag=G, name="zh_sb")
        nc.vector.tensor_tensor(zh_sb[:], z_sb[:], h_sb[:].bitcast(F32), OP.mult)
        nc.vector.tensor_tensor(c_sb[:], zbar_sb[:], c_sb[:], OP.mult)
        nc.vector.tensor_tensor(h_new[:], zh_sb[:], c_sb[:], OP.add)
    else:
        # h == 0:  c = tanh(xh),  h' = (1-z)*c = zbar*c
        for hp in range(2):
            for jj in range(2):
                nc.scalar.activation(c_sb[:, 2 * hp + jj, :], xhh[hp][:, jj, :], AF.Tanh)
        nc.vector.tensor_tensor(h_new[:], zbar_sb[:], c_sb[:], OP.mult)


def _build_bass():
    # Bacc (not raw Bass): its compile() runs move_matmul_waits_to_ldweights
    # + generate_event_semaphores, which split sync waits to satisfy the
    # 1-wait-per-instruction TRN2 constraint walrus enforces.
    nc = bacc.Bacc("TRN2", target_bir_lowering=False, debug=False,
                   enable_asserts=False, num_devices=NCORES)

    dr = {}
    def din(name, shape, dt=F32R):
        dr[name] = nc.dram_tensor(name, list(shape), dt, kind="ExternalInput")
        return dr[name]

    whf = din("whf", (P, KH, H3))
    whb = din("whb", (P, KH, H3))
    whd = din("whd", (P, KH, H3))
    wxf = din("wxf", (P, KE, H3))
    wxb = din("wxb", (P, KE, H3))
    wxd = din("wxd", (P, KE, H3))
    sembT = din("sembT", (P, KE, V))
    tembT = din("tembT", (P, KE, V))
    outw = din("outw", (P, KH, V))
    outb = din("outb", (P, 1), F32)
    ohe = din("ohe", (S, P, BC))
    ohd = din("ohd", (T, P, BC))

    out = nc.dram_tensor("out", [T, P, BC], F32, kind="ExternalOutput")

    with TileContext(nc) as tc:
        with tc.tile_pool(name="cpool", bufs=1) as cpool, \
             tc.tile_pool(name="stdec", bufs=1) as stdec, \
             tc.tile_pool(name="ohp", bufs=4) as ohp:

            # ---- persistent constants ----
            xtabs = {}
            for nm in ("f", "b", "d"):
                xtabs[nm] = cpool.tile([P, H3], F32R, tag="xtab" + nm, name="xtab_" + nm)
            outw_sb = cpool.tile([P, KH, V], F32R, tag="outw", name="outw_sb")
            outb_sb = cpool.tile([P, 1], F32, tag="outb", name="outb_sb")
            # decoder recurrent weights live in cpool so their DMA prefetches
            # during the encoder instead of stalling the phase switch
            whd_sb = cpool.tile([P, KH, H3], F32R, tag="whd", name="whd_sb")

            # decoder initial state halves (persist across phase pools)
            hA0 = stdec.tile([P, KH, BC // 2], F32R, tag="hA0", name="hA0")
            hB0 = stdec.tile([P, KH, BC // 2], F32R, tag="hB0", name="hB0")

            with tc.tile_pool(name="wenc", bufs=1) as wenc, \
                 tc.tile_pool(name="ps", bufs=4, space="PSUM") as ps:

                # ---- build the 3 one-hot projection tables first; their
                # input DMAs are emitted before the big Wh DMAs so each
                # lands on its own HWDGE queue and arrives early ----
                with tc.tile_pool(name="tmpw", bufs=1) as tmpw:
                    # DMA emission order == first-consumer order (the cost
                    # model serializes DMAs, so order is the schedule):
                    # f-table inputs, t0 one-hots, b-table inputs, t1
                    # one-hots, whf (t1 fwd), d-table, whb, whd, out head.
                    tbl = {}
                    def tbl_dma(nm, emb_d, wx):
                        emb_sb = tmpw.tile([P, KE, V], F32R, tag="emb" + nm, name="emb_sb")
                        wx_sb = tmpw.tile([P, KE, H3], F32R, tag="wx" + nm, name="wx_sb")
                        nc.sync.dma_start(out=emb_sb[:], in_=emb_d[:])
                        nc.sync.dma_start(out=wx_sb[:], in_=wx[:])
                        tbl[nm] = (emb_sb, wx_sb)
                    def oh_dma(t):
                        ohf_p = ohp.tile([P, BC], F32R, tag="oh", name="ohf_p")
                        nc.sync.dma_start(out=ohf_p[:], in_=ohe[t])
                        ohb_p = ohp.tile([P, BC], F32R, tag="oh", name="ohb_p")
                        nc.sync.dma_start(out=ohb_p[:], in_=ohe[S - 1 - t])
                        return (ohf_p, ohb_p)

                    tbl_dma("f", sembT, wxf)
                    tbl_dma("b", sembT, wxb)
                    tbl_dma("d", tembT, wxd)
                    oh_pre = [oh_dma(0), oh_dma(1)]
                    whf_sb = wenc.tile([P, KH, H3], F32R, tag="whf", name="whf_sb")
                    whb_sb = wenc.tile([P, KH, H3], F32R, tag="whb", name="whb_sb")
                    nc.sync.dma_start(out=whf_sb[:], in_=whf[:])
                    nc.sync.dma_start(out=whb_sb[:], in_=whb[:])
                    nc.sync.dma_start(out=whd_sb[:], in_=whd[:])
                    tbl_in = [(nm, *tbl[nm]) for nm in ("f", "b", "d")]
                    nc.sync.dma_start(out=outw_sb[:], in_=outw[:])
                    nc.sync.dma_start(out=outb_sb[:], in_=outb[:])

                    for nm, emb_sb, wx_sb in tbl_in:
                        for half in range(2):
                            tp = ps.tile([P, 2, H], F32, tag="ps", name="tp")
                            njj = 2 if half == 0 else 1
                            for jj in range(njj):
                                j_abs = half * 2 + jj
                                for c in range(KE):
                                    nc.tensor.matmul(tp[:, jj, :], emb_sb[:, c, :],
                                                     wx_sb[:, c, j_abs * H:(j_abs + 1) * H],
                                                     start=(c == 0), stop=(c == KE - 1))
                            for jj in range(njj):
                                j_abs = half * 2 + jj
                                nc.vector.tensor_copy(out=xtabs[nm][:, j_abs * H:(j_abs + 1) * H],
                                                      in_=tp[:, jj, :])

                # ---- encoder: fwd + bwd chains interleaved ----
                with tc.tile_pool(name="st", bufs=2) as st, \
                     tc.tile_pool(name="g", bufs=7) as gp:

                    hf = hb = None
                    for t in range(S):
                        last = t == S - 1
                        if t < 2:
                            ohf, ohb = oh_pre[t]
                        else:
                            ohf = ohp.tile([P, BC], F32R, tag="oh", name="ohf")
                            nc.sync.dma_start(out=ohf[:], in_=ohe[t])
                            ohb = ohp.tile([P, BC], F32R, tag="oh", name="ohb")
                            nc.sync.dma_start(out=ohb[:], in_=ohe[S - 1 - t])

                        hf_new = st.tile([P, KH, BC], F32R, tag="hf", name="hf_new")
                        _gru_step(nc, ps, gp, whf_sb, xtabs["f"], ohf[:], hf,
                                  hf_new, BC, "e")
                        hf = hf_new

                        hb_new = st.tile([P, KH, BC], F32R, tag="hb", name="hb_new")
                        _gru_step(nc, ps, gp, whb_sb, xtabs["b"], ohb[:], hb,
                                  hb_new, BC, "e")
                        hb = hb_new

                    # decoder h0 = h_fwd + h_bwd, split into batch halves
                    nc.vector.tensor_tensor(hA0[:], hf[:, :, :BC // 2].bitcast(F32),
                                            hb[:, :, :BC // 2].bitcast(F32), OP.add)
                    nc.vector.tensor_tensor(hB0[:], hf[:, :, BC // 2:].bitcast(F32),
                                            hb[:, :, BC // 2:].bitcast(F32), OP.add)

            # ---- decoder: two batch-half chains interleaved ----
            with tc.tile_pool(name="std", bufs=2) as std, \
                 tc.tile_pool(name="g2", bufs=7) as gp2, \
                 tc.tile_pool(name="lo", bufs=4) as lop, \
                 tc.tile_pool(name="ps2", bufs=8, space="PSUM") as ps2:

                HB = BC // 2

                def emit_logits(h_tile, t, i):
                    # logits = h @ out_W + out_b   (transposed: [V, HB])
                    lp = ps2.tile([P, 2, HB], F32, tag="ps", name="lp")
                    for k in range(KH):
                        nc.tensor.matmul(lp[:, 0, :], outw_sb[:, k, :],
                                         h_tile[:, k, :], start=(k == 0),
                                         stop=(k == KH - 1))
                    lo = lop.tile([P, HB], F32, tag="lo", name="lo")
                    nc.scalar.activation(lo[:], lp[:, 0, :], AF.Identity,
                                         bias=outb_sb[:, 0:1])
                    nc.sync.dma_start(out=out[t, :, i * HB:(i + 1) * HB], in_=lo[:])

                hs = {"A": hA0, "B": hB0}
                # step 0 feeds the unmerged encoder states straight into the
                # matmul accumulation (linearity); the merged hA0/hB0 is only
                # needed late, for the off-path zh term
                hmm0 = {"A": [hf[:, :, :HB], hb[:, :, :HB]],
                        "B": [hf[:, :, HB:], hb[:, :, HB:]]}
                # Logits for a half are emitted only after the OTHER half's
                # next recurrence matmuls, so the PE never queue-blocks on a
                # freshly computed state.
                pend = []  # (h_tile, t, i) awaiting logits
                for t in range(T):
                    oh = ohp.tile([P, BC], F32R, tag="oh", name="oh")
                    nc.sync.dma_start(out=oh[:], in_=ohd[t])
                    for i, half in enumerate(("A", "B")):
                        h_new = std.tile([P, KH, HB], F32R, tag="h" + half, name="h_new")
                        _gru_step(nc, ps2, gp2, whd_sb, xtabs["d"],
                                  oh[:, i * HB:(i + 1) * HB], hs[half], h_new, HB, "d",
                                  h_mm=hmm0[half] if t == 0 else None)
                        hs[half] = h_new
                        if pend:
                            emit_logits(*pend.pop(0))
                        pend.append((h_new, t, i))
                for args in pend:
                    emit_logits(*args)

    nc.compile()
    return nc


def _prep_wh(wh):
    # [H, 3H] -> [128, KH, 3H] with k-tile blocks (partition = row-within-tile)
    return np.ascontiguousarray(
        wh.reshape(KH, P, H3).transpose(1, 0, 2)).astype(np.float32)


def _prep_wx(wx):
    # [E, 3H] -> [128, KE, 3H]
    return np.ascontiguousarray(
        wx.reshape(KE, P, H3).transpose(1, 0, 2)).astype(np.float32)


def _prep_embT(emb):
    # [V, E] -> [128, KE, V]:  embT[p, c, v] = emb[v, c*128+p]
    return np.ascontiguousarray(
        emb.T.reshape(KE, P, V).transpose(1, 0, 2)).astype(np.float32)


def _one_hot(idx_sb):
    # idx_sb: [steps, Bc] int -> [steps, V, Bc] fp32
    steps, bc = idx_sb.shape
    oh = np.zeros((steps, V, bc), np.float32)
    s_ix = np.arange(steps)[:, None]
    b_ix = np.arange(bc)[None, :]
    oh[s_ix, idx_sb, b_ix] = 1.0
    return oh


def kernel(**inputs):
    global LAST_RESULT, _CACHED_NC

    sources = np.asarray(inputs["sources"])
    targets = np.asarray(inputs["targets"])

    # Recurrent biases are structurally zero for this problem (spec
    # fill=zeros); the device program relies on that (out_b IS supported).
    for k in ("enc_fwd_bx", "enc_fwd_bh", "enc_bwd_bx", "enc_bwd_bh",
              "dec_bx", "dec_bh"):
        if np.any(np.asarray(inputs[k]) != 0):
            raise NotImplementedError(f"nonzero bias {k} not supported")

    shared = {
        "whf": _prep_wh(np.asarray(inputs["enc_fwd_Wh"])),
        "whb": _prep_wh(np.asarray(inputs["enc_bwd_Wh"])),
        "whd": _prep_wh(np.asarray(inputs["dec_Wh"])),
        "wxf": _prep_wx(np.asarray(inputs["enc_fwd_Wx"])),
        "wxb": _prep_wx(np.asarray(inputs["enc_bwd_Wx"])),
        "wxd": _prep_wx(np.asarray(inputs["dec_Wx"])),
        "sembT": _prep_embT(np.asarray(inputs["src_emb"])),
        "tembT": _prep_embT(np.asarray(inputs["tgt_emb"])),
        "outw": np.ascontiguousarray(
            np.asarray(inputs["out_W"]).reshape(KH, P, V).transpose(1, 0, 2)
        ).astype(np.float32),
        "outb": np.asarray(inputs["out_b"]).reshape(P, 1).astype(np.float32),
    }

    dec_in = np.concatenate(
        [np.full((B, 1), BOW, dtype=targets.dtype), targets[:, :-1]], axis=1)

    in_maps = []
    for c in range(NCORES):
        sl = slice(c * BC, (c + 1) * BC)
        m = dict(shared)
        m["ohe"] = _one_hot(sources[sl].T)      # [S, V, Bc]
        m["ohd"] = _one_hot(dec_in[sl].T)       # [T, V, Bc]
        in_maps.append(m)

    if _CACHED_NC is None:
        _CACHED_NC = _build_bass()
    nc = _CACHED_NC

    res = run_bass_kernel_spmd(nc, in_maps, core_ids=list(range(NCORES)))
    LAST_RESULT = res

    # gather: per-core staging [T, V, Bc] -> [Bc, T, V]; stack cores on batch
    outs = [np.transpose(r["out"], (2, 0, 1)) for r in res.results]
    return np.ascontiguousarray(np.concatenate(outs, axis=0))
